# revision 7
# baseline (speedup 1.0000x reference)
"""Single-head attention (B=8, S=4096, E=512, H=64) on 8 trn2 NeuronCores.

Sharding: data-parallel over batch — one batch element per core.

Per-core algorithm (batch b), v3:
  - Host pre-transposes x[b] -> xT [E, S] (f32r bits) and converts the
    int32 mask to fp8e4m3 {0,1}, TRANSPOSED and pre-tiled into per-group
    [128, 2, 512] slabs (16 MB vs v1's 32; one contiguous DMA per score
    group, prefetched MLOOK groups ahead so phase B never waits on mask
    DMA — v1 idled the PE ~80us at phase-B start waiting for its big
    mask tiles).
  - QKV: Q^T,K^T [H, S] head-major and V' [S, H+1] S-major (ones column
    appended), all f32r, via PE matmuls over E-chunks.
  - Scores computed TRANSPOSED: S^T[sk, sq] = K^T.T @ Q^T so softmax runs
    along partitions and attn @ V needs no transpose of attn.
  - Mask applied additively PRE-exp with the mask as the MOVING operand
    against a constant stationary -240*I (fp8): sc[:, half] += negI.T @ mT
    = -240 * mT.  One 512-wide matmul per half instead of v1's four
    128-wide matmuls with per-chunk LDWEIGHTS of mask data.
    IMPORTANT dtype note: scores/attn@V stay f32r ON PURPOSE.  An all-
    bf16 phase B streams at the same ns/row when free but trips the HW
    activity throttle (ham k=4 / util limit 0.5) for ~200us; f32r
    streaming does not.  All elementwise two-tensor ops (DVE
    tensor_tensor / scalar_tensor_tensor, Pool tensor_tensor) are
    avoided: they pair-accumulate on lanes 84-95/116-127 under
    concurrent load.
  - exp on ACT with no max-subtraction (|scaled scores| < ~10, safe),
    f32r out; exp(scale*(qk - 240*m)) = 2.5e-5 * w on masked lanes —
    negligible against unmasked softmax mass (rel ~3e-5).
  - Softmax denominator comes free from the ones column of V':
    outT = V'.T @ attn^T accumulates [H+1, sq] where row H is the row sum.
  - Fixup per q block: 4 batched PE transposes into ONE PSUM bank tile,
    reciprocal + scale on DVE (per-partition-scalar ops only), one
    gathered DMA out.

Phase B runs as one flat pipeline over all 128 (qb, g) groups with attn@V
trailing the scores/exp chain by TRAIL groups, so the PE (the bottleneck
engine) never stalls.
"""
import sys

sys.path.insert(0, "/opt/trn_rl_repo")

import ml_dtypes
import numpy as np

import concourse.bacc as bacc
import concourse.tile as tile
from concourse import mybir
from concourse.bass_utils import run_bass_kernel_spmd

F32 = mybir.dt.float32
F32R = mybir.dt.float32r
BF16 = mybir.dt.bfloat16
FP8 = mybir.dt.float8e4

B, S, E, H = 8, 4096, 512, 64
SCALE = float(E) ** -0.5
NEG = -240.0  # max-magnitude finite fp8e4m3 (IEEE): exactly representable

BF16NP = ml_dtypes.bfloat16
FP8NP = ml_dtypes.float8_e4m3

TRAIL = 2   # attn@V trails scores/exp by this many [128,1024] groups
MLOOK = 12  # mask DMA prefetch depth, in groups


def build_program(s=S):
    nc = bacc.Bacc("TRN2", target_bir_lowering=False, debug=False, num_devices=B)
    NE = E // 128          # 4 E-chunks
    NB = s // 512          # q/s blocks of 512
    NQ = s // 128          # 128-row chunks
    NG = NQ // 2           # [128,1024]-score groups per q block
    GQ = NB * NG           # total groups

    xT = nc.dram_tensor("xT", [E, s], F32R, kind="ExternalInput")
    # mask, transposed ([sk, sq]) and pre-tiled per group: row block
    # (g*NB + qb)*128 holds that group's [128, 2, 512] slab contiguously
    maskt = nc.dram_tensor("maskt", [NG * NB * 128, 2, 512], FP8,
                           kind="ExternalInput")
    wq = nc.dram_tensor("wq", [E, H], F32R, kind="ExternalInput")
    wk = nc.dram_tensor("wk", [E, H], F32R, kind="ExternalInput")
    wv = nc.dram_tensor("wv", [E, H], F32R, kind="ExternalInput")
    bqt = nc.dram_tensor("bqt", [H, 1], F32, kind="ExternalInput")
    bkt = nc.dram_tensor("bkt", [H, 1], F32, kind="ExternalInput")
    bv1 = nc.dram_tensor("bv1", [1, H + 1], F32, kind="ExternalInput")
    out = nc.dram_tensor("out", [s, H], F32, kind="ExternalOutput")

    with tile.TileContext(nc) as tc:
        with (
            tc.tile_pool(name="const", bufs=1) as cst,
            tc.tile_pool(name="xp", bufs=2) as xp,
            tc.tile_pool(name="qkv", bufs=1) as qkv,
            tc.tile_pool(name="maskp", bufs=MLOOK) as maskp,
            tc.tile_pool(name="etp", bufs=3) as etp,
            tc.tile_pool(name="osb", bufs=2) as osb,
        ):
            # ---- constants ----
            negI = cst.tile([128, 128], FP8)
            nc.gpsimd.memset(negI, 0.0)
            nc.gpsimd.affine_select(
                out=negI, in_=negI, compare_op=mybir.AluOpType.not_equal,
                fill=NEG, base=0, pattern=[[-1, 128]], channel_multiplier=1,
            )
            idf = cst.tile([128, 128], F32)
            nc.gpsimd.memset(idf, 0.0)
            nc.gpsimd.affine_select(
                out=idf, in_=idf, compare_op=mybir.AluOpType.not_equal,
                fill=1.0, base=0, pattern=[[-1, 128]], channel_multiplier=1,
            )
            ones128 = cst.tile([1, 128], F32)
            nc.vector.memset(ones128, 1.0)

            wq_r = cst.tile([128, NE, H], F32R)
            wk_r = cst.tile([128, NE, H], F32R)
            wv_r = cst.tile([128, NE, H], F32R)
            for w_dram, w_r in ((wq, wq_r), (wk, wk_r), (wv, wv_r)):
                nc.sync.dma_start(
                    out=w_r, in_=w_dram.rearrange("(c p) h -> p c h", p=128)
                )
            bv1_sb = cst.tile([1, H + 1], F32)
            nc.sync.dma_start(out=bv1_sb, in_=bv1[:])
            bqt_sb = cst.tile([H, 1], F32)
            bkt_sb = cst.tile([H, 1], F32)
            nc.sync.dma_start(out=bqt_sb, in_=bqt[:])
            nc.sync.dma_start(out=bkt_sb, in_=bkt[:])

            # ---- mask DMA: per-group [128, 2, 512] fp8 slabs ----
            mtiles = {}

            def mask_dma(G):
                qb, g = divmod(G, NG)
                r0 = (g * NB + qb) * 128
                mt = maskp.tile([128, 2, 512], FP8, tag="mt", name=f"mt_{G}")
                eng = nc.sync if G % 2 == 0 else nc.gpsimd
                eng.dma_start(out=mt, in_=maskt[r0:r0 + 128, :, :])
                mtiles[G] = mt

            for G in range(min(MLOOK, GQ)):
                mask_dma(G)

            # ---- phase A: QT, KT head-major; V' S-major (all f32r) ----
            QTb = [qkv.tile([H, 512], F32R, name=f"qt_{i}") for i in range(NB)]
            KTb = [qkv.tile([H, 512], F32R, name=f"kt_{i}") for i in range(NB)]
            VPk = [qkv.tile([128, H + 1], F32R, name=f"vp_{i}") for i in range(NQ)]
            with tc.tile_pool(name="psA", bufs=2, space="PSUM") as psA:
                for sb in range(NB):
                    s0 = sb * 512
                    xtr = xp.tile([128, NE, 512], F32R, tag="xtr", name=f"xtr_{sb}")
                    half = NE // 2
                    for eh in range(2):
                        e0 = eh * half
                        eng = nc.scalar if eh == 0 else nc.gpsimd
                        eng.dma_start(
                            out=xtr[:, e0:e0 + half, :],
                            in_=xT[e0 * 128:(e0 + half) * 128, s0:s0 + 512]
                            .rearrange("(c p) s -> p c s", p=128),
                        )
                    q_ps = psA.tile([H, 512], F32, tag="qk", name=f"q_ps_{sb}")
                    k_ps = psA.tile([H, 512], F32, tag="qk", name=f"k_ps_{sb}")
                    for e in range(NE):
                        nc.tensor.matmul(q_ps, wq_r[:, e, :], xtr[:, e, :],
                                         start=(e == 0), stop=(e == NE - 1))
                        nc.tensor.matmul(k_ps, wk_r[:, e, :], xtr[:, e, :],
                                         start=(e == 0), stop=(e == NE - 1))
                    nc.scalar.activation(QTb[sb], q_ps,
                                         mybir.ActivationFunctionType.Identity,
                                         bias=bqt_sb)
                    nc.scalar.activation(KTb[sb], k_ps,
                                         mybir.ActivationFunctionType.Identity,
                                         bias=bkt_sb)
                    for j0 in range(0, 4, 2):
                        vps = [
                            psA.tile([128, H + 1], F32, tag="v",
                                     name=f"v_ps_{sb}_{j0 + jj}")
                            for jj in range(2)
                        ]
                        for jj in range(2):
                            nc.tensor.matmul(vps[jj], ones128, bv1_sb,
                                             start=True, stop=False)
                        for e in range(NE):
                            for jj in range(2):
                                c0 = (j0 + jj) * 128
                                nc.tensor.matmul(
                                    vps[jj][:, 0:H], xtr[:, e, c0:c0 + 128],
                                    wv_r[:, e, :], start=False, stop=(e == NE - 1),
                                )
                        for jj in range(2):
                            nc.vector.tensor_copy(VPk[sb * 4 + j0 + jj], vps[jj])

            # ---- phase B: flat pipeline over all (qb, g) groups ----
            with (
                tc.tile_pool(name="psS", bufs=3, space="PSUM") as psS,
                tc.tile_pool(name="psO", bufs=1, space="PSUM") as psO,
            ):
                ot_ps = [None] * NB

                def scores(G):
                    qb, g = divmod(G, NG)
                    sc = psS.tile([128, 1024], F32, tag="sc", name=f"sc_{G}")
                    mt = mtiles.pop(G)
                    for h2 in range(2):
                        k = 2 * g + h2
                        half = sc[:, 512 * h2:512 * h2 + 512]
                        nc.tensor.matmul(
                            half,
                            KTb[k // 4][:, 128 * (k % 4):128 * (k % 4 + 1)],
                            QTb[qb],
                            start=True, stop=False,
                        )
                        nc.tensor.matmul(
                            half, negI, mt[:, h2, :],
                            start=False, stop=True,
                        )
                    return sc

                def expg(G, sc):
                    et = etp.tile([128, 1024], F32R, tag="et")
                    nc.scalar.activation(
                        et, sc, mybir.ActivationFunctionType.Exp, scale=SCALE
                    )
                    return et

                def attnv(G, et):
                    qb, g = divmod(G, NG)
                    if ot_ps[qb] is None:
                        ot_ps[qb] = psO.tile([H + 1, 512], F32, tag="ot",
                                             name=f"ot_{qb}")
                    for h2 in range(2):
                        k = 2 * g + h2
                        nc.tensor.matmul(
                            ot_ps[qb], VPk[k], et[:, 512 * h2:512 * h2 + 512],
                            start=(k == 0), stop=(k == NQ - 1),
                        )

                def fixup(qb):
                    q0 = qb * 512
                    oT = osb.tile([H + 1, 512], F32, tag="oT")
                    nc.vector.tensor_copy(oT, ot_ps[qb])
                    fx = psS.tile([128, 4, H + 1], F32, tag="fx", bufs=1)
                    for j in range(4):
                        nc.tensor.transpose(
                            fx[:, j, :], oT[:, 128 * j:128 * (j + 1)],
                            idf[0:H + 1, 0:H + 1]
                        )
                    ob = osb.tile([128, 4, H + 1], F32, tag="ob")
                    nc.vector.tensor_copy(ob, fx)
                    rc = osb.tile([128, 4], F32, tag="rc")
                    nc.vector.reciprocal(rc, ob[:, :, H])
                    of = osb.tile([128, 4, H], F32, tag="of")
                    for j in range(4):
                        nc.vector.tensor_scalar_mul(
                            of[:, j, :], ob[:, j, 0:H], rc[:, j:j + 1]
                        )
                    nc.gpsimd.dma_start(
                        out=out[q0:q0 + 512, :].rearrange("(j p) h -> p j h", p=128),
                        in_=of,
                    )

                ets = {}
                scn = {0: scores(0)}
                for G in range(GQ):
                    if G + MLOOK < GQ:
                        mask_dma(G + MLOOK)
                    if G + 1 < GQ:
                        scn[G + 1] = scores(G + 1)
                    ets[G] = expg(G, scn.pop(G))
                    if G - TRAIL >= 0:
                        attnv(G - TRAIL, ets.pop(G - TRAIL))
                        if (G - TRAIL) % NG == NG - 1:
                            fixup((G - TRAIL) // NG)
                for G in range(GQ - TRAIL, GQ):
                    attnv(G, ets.pop(G))
                    if G % NG == NG - 1:
                        fixup(G // NG)
    nc.compile()
    return nc


def make_in_maps(x, attention_mask, Wq, bq, Wk, bk, Wv, bv):
    nb = x.shape[0]
    NG, NB = S // 256, S // 512
    bv1 = np.concatenate([bv, np.ones(1, np.float32)]).reshape(1, H + 1)
    common = {
        "wq": np.ascontiguousarray(Wq), "wk": np.ascontiguousarray(Wk),
        "wv": np.ascontiguousarray(Wv),
        "bqt": np.ascontiguousarray(bq.reshape(H, 1)),
        "bkt": np.ascontiguousarray(bk.reshape(H, 1)),
        "bv1": bv1,
    }
    in_maps = []
    for b in range(nb):
        # mask -> fp8 {0,1} bytes (1.0 == 0x38 in e4m3), transposed to
        # [sk, sq], pre-tiled to [(g, qb, p), c, q]
        m8 = ((attention_mask[b] != 0).astype(np.uint8) * np.uint8(0x38))
        mt = m8.T.reshape(NG, 2, 128, NB, 512).transpose(0, 3, 2, 1, 4)
        mt = np.ascontiguousarray(mt).reshape(NG * NB * 128, 2, 512)
        in_maps.append({
            "xT": np.ascontiguousarray(x[b].T),
            "maskt": mt.view(FP8NP),
            **common,
        })
    return in_maps


_PROGRAM = None


def kernel(x, attention_mask, Wq, bq, Wk, bk, Wv, bv):
    global _PROGRAM
    x = np.asarray(x, np.float32)
    attention_mask = np.asarray(attention_mask, np.int32)
    if _PROGRAM is None:
        _PROGRAM = build_program()
    in_maps = make_in_maps(
        x, attention_mask,
        np.asarray(Wq, np.float32), np.asarray(bq, np.float32),
        np.asarray(Wk, np.float32), np.asarray(bk, np.float32),
        np.asarray(Wv, np.float32), np.asarray(bv, np.float32),
    )
    res = run_bass_kernel_spmd(_PROGRAM, in_maps, core_ids=list(range(B)))
    return np.stack([res.results[b]["out"] for b in range(B)], axis=0)


# revision 8
# speedup vs baseline: 1.8625x; 1.8625x over previous
"""Single-head attention (B=8, S=4096, E=512, H=64) on 8 trn2 NeuronCores.

Sharding: data-parallel over batch — one batch element per core.

Per-core algorithm (batch b), v4:
  - Host pre-transposes x[b] -> xT [E, S] in bf16 and converts the int32
    mask to fp8e4m3 {0,1}, pre-tiled into per-group [128, 4, 2, 128]
    slabs (16 MB; one contiguous DMA per score group, prefetched MLOOK
    groups ahead so phase B never waits on mask DMA — v1 idled the PE
    ~80us at phase-B start waiting for its big mask tiles).
  - QKV: Q^T,K^T [H, S] head-major and V' [S, H+1] S-major (ones column
    appended), all bf16, via PE matmuls over E-chunks.
  - Scores computed TRANSPOSED: S^T[sk, sq] = K^T.T @ Q^T so softmax runs
    along partitions and attn @ V needs no transpose of attn.
  - Mask applied additively PRE-exp using the PE's free lhsT transpose
    with the mask chunk STATIONARY and a constant -240*I as the MOVING
    operand: S^T += mask_chunk.T @ (-240 * I), fp8 on both sides.
    POWER NOTE: this shape is chosen deliberately.  The trn2 activity
    governor (ham windows, util limit 0.5 every ~3.4us) clamps the PE to
    ~57% rate when sustained streaming power is too high.  Streaming the
    dense mask as the moving operand (v3) clamps the whole phase B;
    loading the mask as weights and streaming a 99%-zeros diagonal keeps
    window power low.  bf16 (not f32r) scores/attn@V for the same
    reason: f32r streams at ~1.4x the power of bf16 and v3 (f32r) ran
    clamped start to finish at 3411 ns/group vs 1731 free.
    All elementwise two-tensor ops (DVE tensor_tensor /
    scalar_tensor_tensor, Pool tensor_tensor) are avoided: they
    pair-accumulate on lanes 84-95/116-127 under concurrent load.
  - exp on ACT with no max-subtraction (|scaled scores| < ~10, safe),
    bf16 out; exp(scale*(qk - 240*m)) = 2.5e-5 * w on masked lanes —
    negligible against unmasked softmax mass (rel ~3e-5).
  - Softmax denominator comes free from the ones column of V':
    outT = V'.T @ attn^T accumulates [H+1, sq] where row H is the row sum.
  - Fixup per q block: 4 batched PE transposes into ONE PSUM bank tile,
    reciprocal + scale on DVE (per-partition-scalar ops only), one
    gathered DMA out.

Phase B runs as one flat pipeline over all 128 (qb, g) groups with attn@V
trailing the scores/exp chain by TRAIL groups.
"""
import sys

sys.path.insert(0, "/opt/trn_rl_repo")

import ml_dtypes
import numpy as np

import concourse.bacc as bacc
import concourse.tile as tile
from concourse import mybir
from concourse.bass_utils import run_bass_kernel_spmd

F32 = mybir.dt.float32
BF16 = mybir.dt.bfloat16
FP8 = mybir.dt.float8e4

B, S, E, H = 8, 4096, 512, 64
SCALE = float(E) ** -0.5
NEG = -240.0  # max-magnitude finite fp8e4m3 (IEEE): exactly representable

BF16NP = ml_dtypes.bfloat16
FP8NP = ml_dtypes.float8_e4m3

TRAIL = 2   # attn@V trails scores/exp by this many [128,1024] groups
MLOOK = 12  # mask DMA prefetch depth, in groups
IDLE_NS = 0  # deliberate PE idle before phase B (governor credit), ns


def build_program(s=S):
    nc = bacc.Bacc("TRN2", target_bir_lowering=False, debug=False, num_devices=B)
    NE = E // 128          # 4 E-chunks
    NB = s // 512          # q/s blocks of 512
    NQ = s // 128          # 128-row chunks
    NG = NQ // 2           # [128,1024]-score groups per q block
    GQ = NB * NG           # total groups

    xT = nc.dram_tensor("xT", [E, s], BF16, kind="ExternalInput")
    # mask, pre-tiled per group: row block (g*NB + qb)*128 holds that
    # group's [128, 4(j), 2(h2), 128] slab contiguously
    maskt = nc.dram_tensor("maskt", [NG * NB * 128, 4, 2, 128], FP8,
                           kind="ExternalInput")
    wq = nc.dram_tensor("wq", [E, H], BF16, kind="ExternalInput")
    wk = nc.dram_tensor("wk", [E, H], BF16, kind="ExternalInput")
    wv = nc.dram_tensor("wv", [E, H], BF16, kind="ExternalInput")
    bqt = nc.dram_tensor("bqt", [H, 1], F32, kind="ExternalInput")
    bkt = nc.dram_tensor("bkt", [H, 1], F32, kind="ExternalInput")
    bv1 = nc.dram_tensor("bv1", [1, H + 1], BF16, kind="ExternalInput")
    out = nc.dram_tensor("out", [s, H], F32, kind="ExternalOutput")

    with tile.TileContext(nc) as tc:
        with (
            tc.tile_pool(name="const", bufs=1) as cst,
            tc.tile_pool(name="xp", bufs=2) as xp,
            tc.tile_pool(name="qkv", bufs=1) as qkv,
            tc.tile_pool(name="maskp", bufs=MLOOK) as maskp,
            tc.tile_pool(name="etp", bufs=3) as etp,
            tc.tile_pool(name="osb", bufs=2) as osb,
        ):
            # ---- constants ----
            negI = cst.tile([128, 128], FP8)
            nc.gpsimd.memset(negI, 0.0)
            nc.gpsimd.affine_select(
                out=negI, in_=negI, compare_op=mybir.AluOpType.not_equal,
                fill=NEG, base=0, pattern=[[-1, 128]], channel_multiplier=1,
            )
            idf = cst.tile([128, 128], F32)
            nc.gpsimd.memset(idf, 0.0)
            nc.gpsimd.affine_select(
                out=idf, in_=idf, compare_op=mybir.AluOpType.not_equal,
                fill=1.0, base=0, pattern=[[-1, 128]], channel_multiplier=1,
            )
            ones128 = cst.tile([1, 128], BF16)
            nc.vector.memset(ones128, 1.0)

            wq_r = cst.tile([128, NE, H], BF16)
            wk_r = cst.tile([128, NE, H], BF16)
            wv_r = cst.tile([128, NE, H], BF16)
            for w_dram, w_r in ((wq, wq_r), (wk, wk_r), (wv, wv_r)):
                nc.sync.dma_start(
                    out=w_r, in_=w_dram.rearrange("(c p) h -> p c h", p=128)
                )
            bv1_sb = cst.tile([1, H + 1], BF16)
            nc.sync.dma_start(out=bv1_sb, in_=bv1[:])
            bqt_sb = cst.tile([H, 1], F32)
            bkt_sb = cst.tile([H, 1], F32)
            nc.sync.dma_start(out=bqt_sb, in_=bqt[:])
            nc.sync.dma_start(out=bkt_sb, in_=bkt[:])

            # ---- mask DMA: per-group [128, 4, 2, 128] fp8 slabs ----
            mtiles = {}

            def mask_dma(G):
                qb, g = divmod(G, NG)
                r0 = (g * NB + qb) * 128
                mt = maskp.tile([128, 4, 2, 128], FP8, tag="mt", name=f"mt_{G}")
                eng = nc.sync if G % 2 == 0 else nc.gpsimd
                eng.dma_start(out=mt, in_=maskt[r0:r0 + 128, :, :, :])
                mtiles[G] = mt

            for G in range(min(MLOOK, GQ)):
                mask_dma(G)

            # ---- phase A: QT, KT head-major; V' S-major (all bf16) ----
            QTb = [qkv.tile([H, 512], BF16, name=f"qt_{i}") for i in range(NB)]
            KTb = [qkv.tile([H, 512], BF16, name=f"kt_{i}") for i in range(NB)]
            VPk = [qkv.tile([128, H + 1], BF16, name=f"vp_{i}") for i in range(NQ)]
            with tc.tile_pool(name="psA", bufs=2, space="PSUM") as psA:
                for sb in range(NB):
                    s0 = sb * 512
                    xtr = xp.tile([128, NE, 512], BF16, tag="xtr", name=f"xtr_{sb}")
                    half = NE // 2
                    for eh in range(2):
                        e0 = eh * half
                        eng = nc.scalar if eh == 0 else nc.gpsimd
                        eng.dma_start(
                            out=xtr[:, e0:e0 + half, :],
                            in_=xT[e0 * 128:(e0 + half) * 128, s0:s0 + 512]
                            .rearrange("(c p) s -> p c s", p=128),
                        )
                    q_ps = psA.tile([H, 512], F32, tag="qk", name=f"q_ps_{sb}")
                    k_ps = psA.tile([H, 512], F32, tag="qk", name=f"k_ps_{sb}")
                    for e in range(NE):
                        nc.tensor.matmul(q_ps, wq_r[:, e, :], xtr[:, e, :],
                                         start=(e == 0), stop=(e == NE - 1))
                        nc.tensor.matmul(k_ps, wk_r[:, e, :], xtr[:, e, :],
                                         start=(e == 0), stop=(e == NE - 1))
                    nc.scalar.activation(QTb[sb], q_ps,
                                         mybir.ActivationFunctionType.Identity,
                                         bias=bqt_sb)
                    nc.scalar.activation(KTb[sb], k_ps,
                                         mybir.ActivationFunctionType.Identity,
                                         bias=bkt_sb)
                    for j0 in range(0, 4, 2):
                        vps = [
                            psA.tile([128, H + 1], F32, tag="v",
                                     name=f"v_ps_{sb}_{j0 + jj}")
                            for jj in range(2)
                        ]
                        for jj in range(2):
                            nc.tensor.matmul(vps[jj], ones128, bv1_sb,
                                             start=True, stop=False)
                        for e in range(NE):
                            for jj in range(2):
                                c0 = (j0 + jj) * 128
                                nc.tensor.matmul(
                                    vps[jj][:, 0:H], xtr[:, e, c0:c0 + 128],
                                    wv_r[:, e, :], start=False, stop=(e == NE - 1),
                                )
                        for jj in range(2):
                            nc.vector.tensor_copy(VPk[sb * 4 + j0 + jj], vps[jj])

            # ---- optional deliberate idle: bank governor credit ----
            if IDLE_NS > 0:
                # serialized WAW chain on gpsimd; PE's first phase-B matmul
                # depends on negI2 (copy of negI) produced at the end.
                dummy = cst.tile([1, 512], F32)
                n_ops = max(1, IDLE_NS // 450)
                for _ in range(n_ops):
                    nc.gpsimd.memset(dummy, 0.0)

            # ---- phase B: flat pipeline over all (qb, g) groups ----
            with (
                tc.tile_pool(name="psS", bufs=3, space="PSUM") as psS,
                tc.tile_pool(name="psO", bufs=1, space="PSUM") as psO,
            ):
                ot_ps = [None] * NB

                def scores(G):
                    qb, g = divmod(G, NG)
                    sc = psS.tile([128, 1024], F32, tag="sc", name=f"sc_{G}")
                    mt = mtiles.pop(G)
                    for h2 in range(2):
                        k = 2 * g + h2
                        half = sc[:, 512 * h2:512 * h2 + 512]
                        nc.tensor.matmul(
                            half,
                            KTb[k // 4][:, 128 * (k % 4):128 * (k % 4 + 1)],
                            QTb[qb],
                            start=True, stop=False,
                        )
                        for j in range(4):
                            nc.tensor.matmul(
                                half[:, 128 * j:128 * (j + 1)],
                                mt[:, j, h2, :], negI,
                                start=False, stop=(j == 3),
                            )
                    return sc

                def expg(G, sc):
                    et = etp.tile([128, 1024], BF16, tag="et")
                    nc.scalar.activation(
                        et, sc, mybir.ActivationFunctionType.Exp, scale=SCALE
                    )
                    return et

                def attnv(G, et):
                    qb, g = divmod(G, NG)
                    if ot_ps[qb] is None:
                        ot_ps[qb] = psO.tile([H + 1, 512], F32, tag="ot",
                                             name=f"ot_{qb}")
                    for h2 in range(2):
                        k = 2 * g + h2
                        nc.tensor.matmul(
                            ot_ps[qb], VPk[k], et[:, 512 * h2:512 * h2 + 512],
                            start=(k == 0), stop=(k == NQ - 1),
                        )

                def fixup(qb):
                    q0 = qb * 512
                    oT = osb.tile([H + 1, 512], F32, tag="oT")
                    nc.vector.tensor_copy(oT, ot_ps[qb])
                    fx = psS.tile([128, 4, H + 1], F32, tag="fx", bufs=1)
                    for j in range(4):
                        nc.tensor.transpose(
                            fx[:, j, :], oT[:, 128 * j:128 * (j + 1)],
                            idf[0:H + 1, 0:H + 1]
                        )
                    ob = osb.tile([128, 4, H + 1], F32, tag="ob")
                    nc.vector.tensor_copy(ob, fx)
                    rc = osb.tile([128, 4], F32, tag="rc")
                    nc.vector.reciprocal(rc, ob[:, :, H])
                    of = osb.tile([128, 4, H], F32, tag="of")
                    for j in range(4):
                        nc.vector.tensor_scalar_mul(
                            of[:, j, :], ob[:, j, 0:H], rc[:, j:j + 1]
                        )
                    nc.gpsimd.dma_start(
                        out=out[q0:q0 + 512, :].rearrange("(j p) h -> p j h", p=128),
                        in_=of,
                    )

                ets = {}
                scn = {0: scores(0)}
                for G in range(GQ):
                    if G + MLOOK < GQ:
                        mask_dma(G + MLOOK)
                    if G + 1 < GQ:
                        scn[G + 1] = scores(G + 1)
                    ets[G] = expg(G, scn.pop(G))
                    if G - TRAIL >= 0:
                        attnv(G - TRAIL, ets.pop(G - TRAIL))
                        if (G - TRAIL) % NG == NG - 1:
                            fixup((G - TRAIL) // NG)
                for G in range(GQ - TRAIL, GQ):
                    attnv(G, ets.pop(G))
                    if G % NG == NG - 1:
                        fixup(G // NG)
    nc.compile()
    return nc


def make_in_maps(x, attention_mask, Wq, bq, Wk, bk, Wv, bv):
    nb = x.shape[0]
    NG, NB = S // 256, S // 512
    bv1 = np.concatenate([bv, np.ones(1, np.float32)]).reshape(1, H + 1)
    common = {
        "wq": np.ascontiguousarray(Wq.astype(BF16NP)),
        "wk": np.ascontiguousarray(Wk.astype(BF16NP)),
        "wv": np.ascontiguousarray(Wv.astype(BF16NP)),
        "bqt": np.ascontiguousarray(bq.reshape(H, 1)),
        "bkt": np.ascontiguousarray(bk.reshape(H, 1)),
        "bv1": bv1.astype(BF16NP),
    }
    in_maps = []
    for b in range(nb):
        # mask -> fp8 {0,1} bytes (1.0 == 0x38 in e4m3), pre-tiled to
        # [(g, qb, p), j, h2, c]: mask[qb*512 + j*128 + p, (2g+h2)*128 + c]
        m8 = ((attention_mask[b] != 0).astype(np.uint8) * np.uint8(0x38))
        mt = m8.reshape(NB, 4, 128, NG, 2, 128).transpose(3, 0, 2, 1, 4, 5)
        mt = np.ascontiguousarray(mt).reshape(NG * NB * 128, 4, 2, 128)
        in_maps.append({
            "xT": np.ascontiguousarray(x[b].T.astype(BF16NP)),
            "maskt": mt.view(FP8NP),
            **common,
        })
    return in_maps


_PROGRAM = None


def kernel(x, attention_mask, Wq, bq, Wk, bk, Wv, bv):
    global _PROGRAM
    x = np.asarray(x, np.float32)
    attention_mask = np.asarray(attention_mask, np.int32)
    if _PROGRAM is None:
        _PROGRAM = build_program()
    in_maps = make_in_maps(
        x, attention_mask,
        np.asarray(Wq, np.float32), np.asarray(bq, np.float32),
        np.asarray(Wk, np.float32), np.asarray(bk, np.float32),
        np.asarray(Wv, np.float32), np.asarray(bv, np.float32),
    )
    res = run_bass_kernel_spmd(_PROGRAM, in_maps, core_ids=list(range(B)))
    return np.stack([res.results[b]["out"] for b in range(B)], axis=0)
